# revision 17
# baseline (speedup 1.0000x reference)
"""Bayesian linear layer (per-sample weights) on 8 Trainium2 NeuronCores.

out[b,o] = sum_i x[b,i] * (eps[b,i,o]*softplus(ro)[i,o] + mu[i,o])
           + eps_bias[b,o]*softplus(ro_bias)[o] + mu_bias[o]

Strategy (pure i-sharding, transposed accumulation, fp16 eps stream):
  - 8 i-shards: core j owns contraction rows [128j, 128j+128) for ALL 128
    samples; host adds the 8 partial outputs. Replicated mu/ro traffic
    drops to 1MiB/core (vs 4MiB for 2-way i-sharding).
  - eps is host-transposed per core to [i_local(128 partitions), b, o] so
    every DMA has 32KB-contiguous per-partition runs; the 64MiB stream
    goes as 15x 4MiB + 4x 1MiB tiles on the sync HWDGE ring, issued ahead
    of everything else (params ride the scalar ring concurrently).
  - DVE multiplies each sample's [128,1024] slab by softplus(ro) into
    bf16 (fp32/fp32r weight loads run at 4 cyc/col on the PE — 427/214 ns
    per 128-col ldweights+matmul vs 31 ns for bf16 FWL; HW-measured);
    TensorE computes the TRANSPOSED output column: for each
    o-block h, matmul(lhsT=epr[:,128h:128h+128], rhs=x_col[128,1]) writes
    psT[:, 128h+b] (M=128, N=1) — all samples accumulate into ONE
    [128,1024] PSUM tile with no per-sample PSUM drains or row stores.
  - x@mu partials accumulate into a second PSUM tile psmuT in the same
    transposed layout. One final DVE add fuses the two PSUM tiles into
    SBUF; one 512KB store ships out^T (host untransposes).
  - The bias term (eps_bias*softplus(ro_bias)+mu_bias) for the core's
    own 16-sample slice is computed by DVE into a tiny second output
    (64KB); the host adds it while summing the 8 partials (the sample
    offset 16j differs per core and SPMD shares one program, so it can't
    be placed in the shared PSUM column layout on-device).
"""

import numpy as np

import concourse.bass as bass
import concourse.bacc as bacc
import concourse.mybir as mybir
from concourse.tile import TileContext
from concourse.bass_utils import run_bass_kernel_spmd

F32 = mybir.dt.float32
BF16 = mybir.dt.bfloat16
F16 = mybir.dt.float16
AF = mybir.ActivationFunctionType
ALU = mybir.AluOpType

B, IN, OUT = 128, 1024, 1024
NCORES = 8
P = 128
INS = IN // NCORES        # 128 contraction rows per core
NH = OUT // P             # 8 o-blocks
BPC = B // NCORES         # 16 bias samples per core
SPT = 16                  # samples per full eps tile



def build_nc():
    nc = bacc.Bacc(None, target_bir_lowering=False)

    # eps_t[p, b*OUT + o] = eps[b, i0+p, o]
    eps_d = nc.declare_dram_parameter("eps_t", [P, B * OUT], F16, isOutput=False)
    ro_d = nc.declare_dram_parameter("ro", [P, OUT], F32, isOutput=False)
    mu_d = nc.declare_dram_parameter("mu", [P, OUT], F32, isOutput=False)
    # xt[p, b] = x[b, i0+p]
    xt_d = nc.declare_dram_parameter("xt", [P, B], F32, isOutput=False)
    # ebT[p, h*BPC + s] = eps_bias[16j+s, h*128+p]  (this core's 16 samples)
    ebt_d = nc.declare_dram_parameter("ebT", [P, NH * BPC], F32, isOutput=False)
    # rbT[p, h] = ro_bias[0, h*128+p]; mbT likewise
    rbt_d = nc.declare_dram_parameter("rbT", [P, NH], F32, isOutput=False)
    mbt_d = nc.declare_dram_parameter("mbT", [P, NH], F32, isOutput=False)
    # out_t[p, h*B + b] = partial_out[b, h*128+p]
    out_d = nc.declare_dram_parameter("out", [P, NH * B], F16, isOutput=True)
    # bias partial for this core's 16 samples, same transposed layout
    out2_d = nc.declare_dram_parameter("out2", [P, NH * BPC], F16, isOutput=True)

    with TileContext(nc) as tc:
        with (
            tc.tile_pool(name="const", bufs=1) as cpool,
            tc.tile_pool(name="eps", bufs=4) as epool,
            tc.tile_pool(name="epr", bufs=4) as eprpool,
            tc.tile_pool(name="pst", bufs=1, space="PSUM") as pstpool,
            tc.tile_pool(name="psmu", bufs=1, space="PSUM") as pmupool,
        ):
            # ---- tiny/critical params lead the sync ring ----------------
            # xt gates DVE's first (in-order) op and ro gates sigma; on
            # the scalar/Q_X ring they'd starve behind the eps stream.
            ro_t = cpool.tile([P, OUT], F32)
            nc.sync.dma_start(out=ro_t, in_=ro_d[:, :])
            xt = cpool.tile([P, B], F32)
            nc.sync.dma_start(out=xt, in_=xt_d[:, :])
            # scalar-ring params must be issued before the odd eps tiles
            # join that ring, or they queue behind megabytes of eps
            sgb = cpool.tile([P, NH], F32)
            nc.scalar.dma_start(out=sgb, in_=rbt_d[:, :])
            mbt = cpool.tile([P, NH], F32)
            nc.scalar.dma_start(out=mbt, in_=mbt_d[:, :])
            ebt = cpool.tile([P, NH * BPC], F32)
            nc.scalar.dma_start(out=ebt, in_=ebt_d[:, :])
            mt = cpool.tile([P, OUT], F32)
            nc.scalar.dma_start(out=mt, in_=mu_d[:, :])

            # ---- eps stream: issued first, sync ring only ---------------
            # ramp-up: small leading tiles so DVE starts ~5us in; the
            # last tile is split 2-sample sub-DMAs to shorten the tail.
            # chunks: (n_samples, split) covering 128 samples
            CHUNKS = [2, 2, 4, 8] + [16] * 6 + [8, 8]
            assert sum(CHUNKS) == B
            eptiles = []  # (tile, n_samples, base_sample)
            base = 0
            for ci, ns in enumerate(CHUNKS):
                ep = epool.tile([P, SPT * OUT], F16, tag="ep")
                last = ci == len(CHUNKS) - 1
                sub = 2 if last else ns
                ring = nc.sync if ci % 2 == 0 else nc.scalar
                for q0 in range(0, ns, sub):
                    ring.dma_start(
                        out=ep[:, q0 * OUT : (q0 + sub) * OUT],
                        in_=eps_d[
                            :, (base + q0) * OUT : (base + q0 + sub) * OUT
                        ],
                    )
                eptiles.append((ep, ns, base))
                base += ns

            # ---- params on the scalar HWDGE ring ------------------------
            # sigma = softplus(ro), fp16, replicated 4x so mw-sample
            # muls can slice a prefix (each copy is a separate Tile dep).
            # Exp/Ln tables evict each other (~1.28us + engine-0 Q_XIV
            # traffic per load) so batch both Exps, then both Lns.
            nc.scalar.activation(ro_t, ro_t, AF.Exp)
            nc.scalar.activation(sgb, sgb, AF.Exp)
            sig4 = cpool.tile([P, 4 * OUT], F16)
            nc.scalar.activation(sig4[:, :OUT], ro_t, AF.Ln, bias=1.0)
            nc.scalar.activation(sgb, sgb, AF.Ln, bias=1.0)
            for r in range(1, 4):
                nc.scalar.copy(
                    sig4[:, r * OUT : (r + 1) * OUT], sig4[:, :OUT]
                )

            xtb = cpool.tile([P, B], F16)
            nc.vector.tensor_copy(out=xtb, in_=xt)

            # ---- x @ mu partial (transposed) ----------------------------
            psmu = pmupool.tile([P, OUT], F32)
            for h in range(NH):
                nc.tensor.matmul(
                    psmu[:, h * B : (h + 1) * B],
                    mt[:, h * P : (h + 1) * P],
                    xt[:, :],
                    start=True,
                    stop=True,
                )

            # psmu is done early; park it in SBUF so the final DVE add
            # reads only one PSUM operand (HW limit).
            pmsb = cpool.tile([P, OUT], F32)
            nc.scalar.copy(pmsb, psmu)

            # ---- main streaming loop ------------------------------------
            psT = pstpool.tile([P, OUT], F32)
            last_ci = len(eptiles) - 1
            for ci, (ep, ns, base) in enumerate(eptiles):
                # ramp mul width with sig4 fill; narrow at the tail
                mw = [1, 2][ci] if ci < 2 else (2 if ci == last_ci else 4)
                for k0 in range(0, ns, mw):
                    epr = eprpool.tile([P, 4 * OUT], F16, tag="epr")
                    nc.vector.tensor_mul(
                        out=epr[:, : mw * OUT],
                        in0=ep[:, k0 * OUT : (k0 + mw) * OUT],
                        in1=sig4[:, : mw * OUT],
                    )
                    for s in range(mw):
                        b = base + k0 + s
                        for h in range(NH):
                            nc.tensor.matmul(
                                psT[:, h * B + b : h * B + b + 1],
                                epr[:, s * OUT + h * P : s * OUT + (h + 1) * P],
                                xtb[:, b : b + 1],
                                start=True,
                                stop=True,
                            )

            # ---- bias partial: out2 = ebT*softplus(rbT) + mbT -----------
            # placed after the muls so the DVE in-order stream never
            # stalls mid-pipeline waiting for sgb/mbt
            o2sb = cpool.tile([P, NH * BPC], F16)
            for h in range(NH):
                nc.vector.tensor_scalar(
                    out=o2sb[:, h * BPC : (h + 1) * BPC],
                    in0=ebt[:, h * BPC : (h + 1) * BPC],
                    scalar1=sgb[:, h : h + 1],
                    scalar2=mbt[:, h : h + 1],
                    op0=ALU.mult,
                    op1=ALU.add,
                )
            nc.scalar.dma_start(out=out2_d[:, :], in_=o2sb)

            # ---- fuse + single store ------------------------------------
            osb = cpool.tile([P, OUT], F16)
            nc.vector.tensor_add(out=osb, in0=psT, in1=pmsb)
            nc.sync.dma_start(out=out_d[:, :], in_=osb)

    nc.finalize()
    return nc


_NC_CACHE = None


def _get_nc():
    global _NC_CACHE
    if _NC_CACHE is None:
        _NC_CACHE = build_nc()
    return _NC_CACHE


def kernel(x, mu, ro, mu_bias, ro_bias, eps, eps_bias, _trace=False, _tmpdir=None):
    x = np.asarray(x, dtype=np.float32)
    mu = np.asarray(mu, dtype=np.float32)
    ro = np.asarray(ro, dtype=np.float32)
    mu_bias = np.asarray(mu_bias, dtype=np.float32).reshape(1, OUT)
    ro_bias = np.asarray(ro_bias, dtype=np.float32).reshape(1, OUT)
    eps = np.asarray(eps, dtype=np.float32)
    eps_bias = np.asarray(eps_bias, dtype=np.float32)

    nc = _get_nc()

    # rbT[p, h] = ro_bias[0, h*128+p]
    rbt = np.ascontiguousarray(ro_bias.reshape(NH, P).T)
    mbt = np.ascontiguousarray(mu_bias.reshape(NH, P).T)

    in_maps = []
    for j in range(NCORES):
        i0 = j * INS
        eps_t = np.ascontiguousarray(
            eps[:, i0 : i0 + INS, :].transpose(1, 0, 2).astype(np.float16)
        ).reshape(P, B * OUT)
        # ebT[p, h*16+s] = eps_bias[16j+s, h*128+p]
        ebt = np.ascontiguousarray(
            eps_bias[j * BPC : (j + 1) * BPC].reshape(BPC, NH, P).transpose(2, 1, 0)
        ).reshape(P, NH * BPC)
        in_maps.append(
            {
                "eps_t": eps_t,
                "ro": np.ascontiguousarray(ro[i0 : i0 + INS, :]),
                "mu": np.ascontiguousarray(mu[i0 : i0 + INS, :]),
                "xt": np.ascontiguousarray(x[:, i0 : i0 + INS].T),
                "ebT": ebt,
                "rbT": rbt,
                "mbT": mbt,
            }
        )

    res = run_bass_kernel_spmd(
        nc, in_maps, core_ids=list(range(NCORES)), trace=_trace, tmpdir=_tmpdir
    )
    # out_t[p, h*B + b] -> out[b, h*128+p]; sum partials over cores
    acc = res.results[0]["out"].astype(np.float32)
    for j in range(1, NCORES):
        acc += res.results[j]["out"].astype(np.float32)
    out = np.ascontiguousarray(
        acc.reshape(P, NH, B).transpose(2, 1, 0).reshape(B, OUT)
    )
    # add per-core bias partials: out2[p, h*BPC+s] -> out[16j+s, h*128+p]
    for j in range(NCORES):
        o2 = res.results[j]["out2"].astype(np.float32)
        out[j * BPC : (j + 1) * BPC] += o2.reshape(P, NH, BPC).transpose(
            2, 1, 0
        ).reshape(BPC, OUT)
    if _trace:
        kernel.last_results = res
    return out
